# revision 1
# baseline (speedup 1.0000x reference)
"""GCN layer (2-hop SpMM + per-hop Linear/ReLU) on 8 Trainium2 NeuronCores.

Strategy (dst-sharded graph parallel, per the sharding hint):
  - Nodes sharded 1250/core; each core owns the edges pointing at its shard.
  - Host sorts edges by (dst block, src), packs them into 128-edge chunks per
    128-dst block, and builds per-chunk one-hot scatter matrices S
    (S[p,m] = w_e for edge p landing on local dst m) plus dma_gather index
    lists of src ids. S and the chunk structure are shared by both hops.
  - Per hop: dma_gather pulls h[src] rows (bf16) from a per-core DRAM copy
    into SBUF G tiles [128 edges, 512 feat]; TensorE computes
    psum += S.T @ G per chunk (the scatter-add); ScalarE evicts with the
    per-dst D_norm scale (bf16); HWDGE DMA-transpose builds the feat-major
    copy used by the linear layers.
  - Hop-1 blocks are broadcast with per-block AllGathers into a
    block-interleaved DRAM layout (row bl*1024 + core*128 + p) as soon as
    each block is evicted. Hop-2 edges are src-sorted, so each hop-2 gather
    call only reads a PREFIX of that layout — Tile's dependency tracking
    then pipelines hop 2 into hop 1 instead of waiting for a full
    all-gather barrier.
  - Linear stage runs feat-major: outT[fo, n] = relu(W.T @ hT + b), bias and
    relu fused in one ScalarE activation (bias is per-partition there).
    The [1536, 1250] per-core outputs are concatenated + transposed on host.
"""

import sys

sys.path.insert(0, "/opt/trn_rl_repo")

import numpy as np
import ml_dtypes

import concourse.bass as bass
import concourse.bacc as bacc
import concourse.mybir as mybir
import concourse.tile as tile
from concourse import library_config
from concourse.bass_utils import run_bass_kernel_spmd

N_NODES = 10000
N_EDGES = 160000
D = 512
ORDER = 2
N_CORES = 8
SHARD = N_NODES // N_CORES          # 1250
BLKS = (SHARD + 127) // 128         # 10 dst blocks per core
BLK_SZ = [min(128, SHARD - b * 128) for b in range(BLKS)]  # [128]*9 + [98]
FI = D // 128                       # 4 feat-in chunks
FO = D // 128                       # 4 feat-out tiles
NGRPS = [512, 512, SHARD - 1024]    # node groups for linear stage
GSPLIT = 2                          # gather calls per (block, hop)
COLL_GRP = 5                        # dst blocks per all-gather collective
BF16 = ml_dtypes.bfloat16


def _split_excess_waits(nc, max_waits=1):
    """This walrus build rejects >1 sync wait per instruction (and any on a
    Drain). Hoist excess SyncWaits onto InstNoOp carriers inserted just
    before, on the same engine — waits execute in program order, so
    semantics are preserved."""
    for fn in nc.m.functions:
        for bb in fn.blocks:
            new = []
            changed = False
            for inst in bb.instructions:
                si = inst.sync_info
                cap = 0 if isinstance(inst, mybir.InstDrain) else max_waits
                if si is not None and len(si.on_wait) > cap:
                    waits = list(si.on_wait)
                    excess = waits[:-cap] if cap else waits
                    keep = waits[-cap:] if cap else []
                    for g in range(0, len(excess), max_waits):
                        nop = mybir.InstNoOp(name=f"{inst.name}-ws{g}", ins=[], outs=[])
                        nop.engine = inst.engine
                        nop.sync_info = mybir.SyncInfo(
                            on_wait=excess[g:g + max_waits], on_update=[])
                        new.append(nop)
                    si.on_wait = keep
                    changed = True
                new.append(inst)
            if changed:
                bb.instructions = new


def _assign_gather_queues(nc):
    """Tile locks each DMASW sem lane to SWDGE queue lane%4; route every
    gather through the queue matching its (scheduler-assigned) sem lane so
    the 4 SWDGE queues actually run in parallel."""
    for fn in nc.m.functions:
        for bb in fn.blocks:
            for inst in bb.instructions:
                if isinstance(inst, mybir.InstDMAGatherAnt):
                    si = inst.sync_info
                    if not si:
                        continue
                    for u in si.on_update:
                        nm = getattr(u, "ant_name", "") or ""
                        if nm.startswith("DMASW"):
                            lane = int(nm[5:].split("_")[0])
                            inst.queue_num = lane % 4
                            break


def _blocked_row(g):
    """Row of global node g in the grouped all-gather output layout:
    group grp = bl//COLL_GRP gathers [core j][block-in-group b][p]."""
    j, l = g // SHARD, g % SHARD
    bl, p = l // 128, l % 128
    grp, b = bl // COLL_GRP, bl % COLL_GRP
    return grp * (N_CORES * COLL_GRP * 128) + j * (COLL_GRP * 128) + b * 128 + p


def _preprocess(features, D_norm, edge_w, W, b, src, dst):
    """Host-side: shard edges by dst owner, sort by (dst block, src),
    chunk, build S and both hops' gather index tensors."""
    core_of = dst // SHARD
    per_core = []
    for i in range(N_CORES):
        sel = np.nonzero(core_of == i)[0]
        dl = dst[sel] - i * SHARD
        order = np.lexsort((_blocked_row(src[sel].astype(np.int64)), dl // 128))
        per_core.append((sel[order], dl[order]))

    nchk = np.zeros(BLKS, np.int64)
    for i in range(N_CORES):
        _, dl = per_core[i]
        cnt = np.bincount(dl // 128, minlength=BLKS)
        nchk = np.maximum(nchk, (cnt + 127) // 128)
    nchk = np.maximum(nchk, 1).astype(np.int64)
    ncht = int(nchk.sum())

    idx1 = np.zeros((N_CORES, 128, ncht * 8), np.int16)
    idx2 = np.zeros((N_CORES, 128, ncht * 8), np.int16)
    s_t = np.zeros((N_CORES, 128, ncht, 128), np.float32)
    blk_chunk_off = np.concatenate([[0], np.cumsum(nchk)])

    # split_chunk[bi]: chunks [0, split) of block bi only touch collective
    # group 0 rows on every core (prefix 5); the rest need the full prefix
    split_chunk = nchk.copy()

    for i in range(N_CORES):
        eids, dl = per_core[i]
        w = edge_w[eids]
        s = src[eids]
        blk = dl // 128
        m = dl - blk * 128
        cnt = np.bincount(blk, minlength=BLKS)
        boff = np.concatenate([[0], np.cumsum(cnt)])[:-1]
        pos = np.arange(len(eids)) - boff[blk]
        chunk = blk_chunk_off[blk] + pos // 128
        lane = pos % 128
        s_t[i, lane, chunk, m] = w
        lin = chunk * 128 + lane
        f1 = np.zeros(ncht * 128, np.int16)
        f1[lin] = s.astype(np.int16)
        sb = _blocked_row(s.astype(np.int64))
        f2 = np.zeros(ncht * 128, np.int16)
        f2[lin] = sb.astype(np.int16)
        idx1[i] = np.tile(f1.reshape(-1, 16).T, (8, 1))
        idx2[i] = np.tile(f2.reshape(-1, 16).T, (8, 1))
        # last chunk of each block whose rows stay within collective group 0
        grp_rows = N_CORES * COLL_GRP * 128
        for bi in range(BLKS):
            nch = int(nchk[bi])
            lo = int(blk_chunk_off[bi])
            rows = f2[lo * 128:(lo + nch) * 128].reshape(nch, 128)
            ok = int((rows.max(axis=1) < grp_rows).cumprod().sum())
            split_chunk[bi] = min(split_chunk[bi], ok)

    return nchk, ncht, idx1, idx2, s_t.astype(BF16), split_chunk


def _build_program(nchk, ncht, split_chunk, split_waits=True):
    nc = bacc.Bacc("TRN2", num_swdge_queues=4)
    dt = mybir.dt

    h0_full = nc.declare_dram_parameter("h0_full", [N_NODES, D], dt.bfloat16, isOutput=False)
    h0t_shard = nc.declare_dram_parameter("h0t_shard", [128, FI, SHARD], dt.bfloat16, isOutput=False)
    idx1_in = nc.declare_dram_parameter("idx1", [128, ncht * 8], dt.int16, isOutput=False)
    idx2_in = nc.declare_dram_parameter("idx2", [128, ncht * 8], dt.int16, isOutput=False)
    s_in = nc.declare_dram_parameter("s", [128, ncht, 128], dt.bfloat16, isOutput=False)
    d_in = nc.declare_dram_parameter("dnorm", [128, BLKS], dt.float32, isOutput=False)
    w_in = nc.declare_dram_parameter("w", [128, ORDER + 1, FI, D], dt.bfloat16, isOutput=False)
    b_in = nc.declare_dram_parameter("bias", [128, ORDER + 1, FO], dt.float32, isOutput=False)
    out_t = nc.declare_dram_parameter("out_t", [(ORDER + 1) * D, SHARD], dt.float32, isOutput=True)

    # shard bounce (block bi rows at bi*128) + grouped-gather h1 layout
    h1_shard_dram = nc.dram_tensor("h1_shard", [BLKS * 128, D], dt.bfloat16)
    h1_blocked = nc.dram_tensor("h1_blocked", [BLKS * 1024, D], dt.bfloat16,
                                addr_space="Shared")

    blk_off = np.concatenate([[0], np.cumsum(nchk)])
    qctr = [0]

    with tile.TileContext(nc) as tc:
        nc.gpsimd.load_library(library_config.mlp)
        with (
            tc.tile_pool(name="const", bufs=1) as const,
            tc.tile_pool(name="gbuf", bufs=6) as gbuf,
            tc.tile_pool(name="evict", bufs=3) as evict,
            tc.tile_pool(name="lin", bufs=3) as lin,
            tc.tile_pool(name="psum", bufs=4, space=bass.MemorySpace.PSUM) as psum,
            tc.tile_pool(name="psw", bufs=4, space=bass.MemorySpace.PSUM) as psw,
        ):
            idx1_t = const.tile([128, ncht * 8], dt.int16)
            nc.sync.dma_start(idx1_t[:], idx1_in[:])
            idx2_t = const.tile([128, ncht * 8], dt.int16)
            nc.sync.dma_start(idx2_t[:], idx2_in[:])
            s_t = const.tile([128, ncht, 128], dt.bfloat16)
            nc.sync.dma_start(s_t[:], s_in[:])
            d_t = const.tile([128, BLKS], dt.float32)
            nc.sync.dma_start(d_t[:], d_in[:])
            w_t = const.tile([128, ORDER + 1, FI, D], dt.bfloat16)
            nc.sync.dma_start(w_t[:], w_in[:])
            b_t = const.tile([128, ORDER + 1, FO], dt.float32)
            nc.sync.dma_start(b_t[:], b_in[:])

            # feat-major hop results; hT[p, f, n] = h[n, f*128+p]
            # (free dim padded to BLKS*128 so the last block's transpose fits)
            ht = [const.tile([128, FI, BLKS * 128], dt.bfloat16, tag=f"ht{k}",
                             name=f"ht{k}")
                  for k in range(ORDER + 1)]
            nc.sync.dma_start(ht[0][:, :, :SHARD], h0t_shard[:])

            def hop_block(bi, k, idx_t, src_view):
                """One dst block of one SpMM hop. src_view(bi, gsp) -> in_ap
                for that gather call (prefix-sliced for hop 2)."""
                nch = int(nchk[bi])
                off = int(blk_off[bi])
                sp = int(split_chunk[bi]) if k == 2 else (nch + 1) // 2
                cuts = [c for c in (0, sp, nch) if 0 <= c <= nch]
                cuts = sorted(set(cuts))
                g = gbuf.tile([128, nch, D], dt.bfloat16, tag="g", name="g")
                for gsp in range(len(cuts) - 1):
                    c0, c1 = cuts[gsp], cuts[gsp + 1]
                    nc.gpsimd.dma_gather(
                        out_ap=g[:, c0:c1, :],
                        in_ap=src_view(bi, gsp),
                        idxs_ap=idx_t[:, (off + c0) * 8:(off + c1) * 8],
                        num_idxs=(c1 - c0) * 128,
                        num_idxs_reg=(c1 - c0) * 128,
                        elem_size=D,
                        single_packet=False,
                    )
                    qctr[0] += 1
                acc = psum.tile([128, D], dt.float32, tag="agg", name="acc")
                for c in range(nch):
                    nc.tensor.matmul(acc[:], s_t[:, off + c, :], g[:, c, :],
                                     start=(c == 0), stop=(c == nch - 1))
                hb = evict.tile([128, D], dt.bfloat16, tag="hb", name="hb")
                nc.scalar.activation(
                    out=hb[:], in_=acc[:],
                    func=mybir.ActivationFunctionType.Copy,
                    scale=d_t[:, bi:bi + 1])
                nc.sync.dma_start_transpose(
                    ht[k][:, :, bi * 128:bi * 128 + 128], hb[:])
                return hb

            def linear(k):
                for ft in range(FO):
                    for gi, gsz in enumerate(NGRPS):
                        goff = sum(NGRPS[:gi])
                        pw = psw.tile([128, gsz], dt.float32, tag="pw", name="pw")
                        for fi in range(FI):
                            nc.tensor.matmul(
                                pw[:], w_t[:, k, fi, ft * 128:(ft + 1) * 128],
                                ht[k][:, fi, goff:goff + gsz],
                                start=(fi == 0), stop=(fi == FI - 1))
                        ob = lin.tile([128, gsz], dt.float32, tag="ob", name="ob")
                        nc.scalar.activation(
                            out=ob[:], in_=pw[:],
                            func=mybir.ActivationFunctionType.Relu,
                            bias=b_t[:, k, ft:ft + 1])
                        nc.sync.dma_start(
                            out_t[k * D + ft * 128:k * D + (ft + 1) * 128,
                                  goff:goff + gsz], ob[:])

            linear(0)

            # hop 1: gather from the full h0 copy; broadcast each block as
            # soon as it is evicted
            h0_view = lambda bi, gsp: h0_full[:]
            for bi in range(BLKS):
                hb = hop_block(bi, 1, idx1_t, h0_view)
                nc.sync.dma_start(
                    h1_shard_dram[bi * 128:(bi + 1) * 128, :], hb[:])
                if (bi + 1) % COLL_GRP == 0:
                    g0 = bi + 1 - COLL_GRP
                    # issued on vector so the serialized ncfw collectives
                    # don't block the gpsimd gather queue
                    nc.gpsimd.collective_compute(
                        "AllGather",
                        mybir.AluOpType.bypass,
                        replica_groups=[list(range(N_CORES))],
                        ins=[h1_shard_dram[g0 * 128:(bi + 1) * 128, :]],
                        outs=[h1_blocked[g0 * 1024:(bi + 1) * 1024, :]],
                    )

            # hop 2: src-sorted chunks read only a prefix of the broadcast
            # blocks, so these gathers pipeline into hop 1's collectives
            h1_view = lambda bi, gsp: h1_blocked[
                :(COLL_GRP if gsp == 0 else BLKS) * 1024, :]
            
            for bi in range(BLKS):
                hop_block(bi, 2, idx2_t, h1_view)

            linear(1)
            linear(2)

    nc.compile()
    _assign_gather_queues(nc)
    if split_waits:
        _split_excess_waits(nc)
    return nc


def kernel(features, D_norm, edge_w, W, b, src, dst, _timing=None):
    features = np.asarray(features, np.float32)
    D_norm = np.asarray(D_norm, np.float32)
    edge_w = np.asarray(edge_w, np.float32)
    W = np.asarray(W, np.float32)
    b = np.asarray(b, np.float32)
    src = np.asarray(src, np.int32)
    dst = np.asarray(dst, np.int32)

    nchk, ncht, idx1, idx2, s_t, split_chunk = _preprocess(
        features, D_norm, edge_w, W, b, src, dst)
    nc = _build_program(nchk, ncht, split_chunk)

    h0_bf = features.astype(BF16)
    w_pack = np.zeros((128, ORDER + 1, FI, D), np.float32)
    for fi in range(FI):
        w_pack[:, :, fi, :] = W[:, fi * 128:(fi + 1) * 128, :].transpose(1, 0, 2)
    b_pack = np.zeros((128, ORDER + 1, FO), np.float32)
    for ft in range(FO):
        b_pack[:, :, ft] = b[:, ft * 128:(ft + 1) * 128].T

    in_maps = []
    for i in range(N_CORES):
        sh = slice(i * SHARD, (i + 1) * SHARD)
        h0t = features[sh].reshape(SHARD, FI, 128).transpose(2, 1, 0)
        dp = np.zeros((128, BLKS), np.float32)
        dflat = D_norm[sh, 0]
        for bi in range(BLKS):
            dp[:BLK_SZ[bi], bi] = dflat[bi * 128:bi * 128 + BLK_SZ[bi]]
        in_maps.append({
            "h0_full": h0_bf,
            "h0t_shard": h0t.astype(BF16).copy(),
            "idx1": idx1[i],
            "idx2": idx2[i],
            "s": s_t[i],
            "dnorm": dp,
            "w": w_pack.astype(BF16),
            "bias": b_pack,
        })

    res = run_bass_kernel_spmd(
        nc, in_maps, list(range(N_CORES)),
        trace=bool(_timing is not None))
    if _timing is not None:
        _timing["exec_time_ns"] = res.exec_time_ns

    parts = [np.asarray(res.results[i]["out_t"]) for i in range(N_CORES)]
    out = np.concatenate(parts, axis=1).T          # [N, 3*D]
    return np.ascontiguousarray(out, dtype=np.float32)

